# revision 15
# baseline (speedup 1.0000x reference)
"""Trainium2 Bass kernel for nn_NovaLinkPredictor (hetero GraphSAGE link predictor).

8-core SPMD, v2:
  - Stage0: movie_x/movie_h/xcat tables per movie slice (bf16), biases folded into
    the tables (B1=bl1_mu+u0@Wr1_mu into p1, B2=bl2_mu into p2, B3=bl2_um into userh).
  - Pass B (user-side aggregation): edges sharded by src user range, ordered
    (cohort-of-10-user-tiles, movie-range, tile, dst); per-chunk one-hot scatter
    matmuls accumulate straight in PSUM across all 3 movie ranges (no SBUF acc).
    Degrees/recips come from host bincount; zero-degree users get fake edges to a
    patched [B1|B2] table row so mean+bias stays exact.
  - Pass C (movie-side aggregation): edges sharded by dst movie range; gathers the
    AllGather'd userh table (user_h @ Wl2_um + B3); direct local accumulation
    (no AllToAll / partials round-trip).
  - Pass D: labels sharded by user range; user_o rows come from SBUF-resident
    user_o via one-hot expansion matmuls (no user-side gather); movie rows via
    dma_gather of mo_full; dots via scalar_tensor_tensor accumulate.
  - Collectives: 3 AllGathers (xcat 5.2MB/rank, userh 6.4MB/rank, mo 2.6MB/rank).
"""
import sys
sys.path.insert(0, "/opt/trn_rl_repo")
import numpy as np
import ml_dtypes

from concourse import bass, mybir, bacc, tile
from concourse.bass_utils import run_bass_kernel_spmd
from concourse.masks import make_identity

# ---------------- constants ----------------
H = 128
NU = 200000
NM = 80000
FD = 512
W = 8
P = 128

USR = 25088            # users per core (196 tiles)
UT = 196
NUP = USR * W          # 200704
MSL = 10112            # movies per core (79 tiles)
MT = 79
NMP = MSL * W          # 80896

# split-AG table geometry: each global table is AllGather'd in two halves
# (A = early tiles, B = late tiles) so the collective overlaps the producer
# pass; gather indices are remapped host-side into block-concat layouts.
XC_A = 5120            # xcat: local rows in half A (tiles 0..39)
XC_B = MSL - XC_A      # 4992 (tiles 40..78)
XR = [(0, 0, 20480), (0, 20480, 20480), (1, 0, 19968), (1, 19968, 19968)]
NRM = 4                # pass B / xcat ranges (2 per half)
UH_A = 12288           # userh: half A local rows (tiles 0..95)
UH_B = USR - UH_A      # 12800 (tiles 96..195)
UR = [(0, k * 24576, 24576) for k in range(4)] + [(1, k * 25600, 25600) for k in range(4)]
NRU = 8
MO_A = 5376            # mo: half A local rows (tiles 0..41)
MO_B = MSL - MO_A      # 4736
MR = [(0, 0, 21504), (0, 21504, 21504), (1, 0, 18944), (1, 18944, 18944)]
NRD = 4
ZROW_M = 80000         # global movie row patched to [B1|B2] (core 7 local 9216)
ZROW_U = 200000        # global user row whose userh row is B3 (pad user)
SENT = 200.0
COH = 6                # user/movie tiles per psum cohort (6 PSUM banks + 2 epilogue)
GROUP = 8              # chunks per dma_gather call (>1024 idx crashes HW)

bf16 = mybir.dt.bfloat16
f32 = mybir.dt.float32
i16 = mybir.dt.int16
npbf16 = ml_dtypes.bfloat16
AF = mybir.ActivationFunctionType


# ---------------- host-side preprocessing ----------------

def _wrap16(idx):
    n = idx.shape[0]
    assert n % 16 == 0
    w = idx.reshape(n // 16, 16).T.astype(np.int16)
    return np.ascontiguousarray(np.tile(w, (8, 1)))


def _col_layout(vals, n_chunks, fill, dtype):
    a = np.full(n_chunks * P, fill, dtype=dtype)
    a[: len(vals)] = vals
    return np.ascontiguousarray(a.reshape(n_chunks, P).T)


def _build_pass_streams(core_of, loc_of, gi_of, rng_of, tile_of, n_rng, n_tiles,
                        coh, min1_r0=True):
    """Generic (cohort, range, tile)-ordered stream builder.

    core_of/loc_of/gi_of/rng_of/tile_of: per-edge arrays (already restricted is
    done by caller via boolean masks per core).
    Returns (chunks[r][t] padded-to-max array, per-core (gidx_stream, loc_stream)).
    """
    cnt = np.zeros((W, n_rng, n_tiles), np.int64)
    np.add.at(cnt, (core_of, rng_of, tile_of), 1)
    mx = cnt.max(axis=0)
    chunks = (mx + P - 1) // P
    if min1_r0:
        chunks[0] = np.maximum(chunks[0], 1)
    return chunks, cnt


def _split_lut(n_total, per_core, a_rows, ranges):
    """Map global id -> (range_id, local_idx) for a 2-half block-concat table."""
    ids = np.arange(n_total, dtype=np.int64)
    c = ids // per_core
    l = ids - c * per_core
    b_rows = per_core - a_rows
    in_a = l < a_rows
    row = np.where(in_a, c * a_rows + l, c * b_rows + (l - a_rows))
    half = (~in_a).astype(np.int64)
    rng = np.full(n_total, -1, np.int64)
    loc = np.zeros(n_total, np.int64)
    for ri, (h, st, ln) in enumerate(ranges):
        m = (half == h) & (row >= st) & (row < st + ln)
        rng[m] = ri
        loc[m] = row[m] - st
    assert (rng >= 0).all()
    return rng, loc


def preprocess(edge_src, edge_dst, lbl_user, lbl_movie):
    S = {}
    es = np.asarray(edge_src).astype(np.int64)
    ed = np.asarray(edge_dst).astype(np.int64)
    lu = np.asarray(lbl_user).astype(np.int64)
    lm = np.asarray(lbl_movie).astype(np.int64)

    udeg = np.bincount(es, minlength=NUP)
    mdeg = np.bincount(ed, minlength=NMP)
    recip_u = 1.0 / np.maximum(udeg, 1).astype(np.float64)
    recip_m = 1.0 / np.maximum(mdeg, 1).astype(np.float64)
    S["recip_u"] = [
        np.ascontiguousarray(
            recip_u[c * USR:(c + 1) * USR].reshape(UT, P).T.astype(np.float32))
        for c in range(W)]
    S["recip_m"] = [
        np.ascontiguousarray(
            recip_m[c * MSL:(c + 1) * MSL].reshape(MT, P).T.astype(np.float32))
        for c in range(W)]

    # fake edges: zero-degree users -> ZROW_M ([B1|B2] row); zero-deg movies -> ZROW_U
    fu = np.nonzero(udeg[:NU] == 0)[0]
    fm = np.nonzero(mdeg[:NM] == 0)[0]
    es_a = np.concatenate([es, fu, np.full(len(fm), ZROW_U, np.int64)])
    ed_a = np.concatenate([ed, np.full(len(fu), ZROW_M, np.int64), fm])

    # ---- pass B: shard by src core; order (cohort, r, tile, dst) ----
    u_core = es_a // USR
    u_loc = es_a - u_core * USR
    b_tile = u_loc // P
    _xr, _xl = _split_lut(NMP, MSL, XC_A, XR)
    b_rng = _xr[ed_a]
    b_gi = _xl[ed_a]
    chunksB, cntB = _build_pass_streams(u_core, None, None, b_rng, b_tile, NRM, UT, COH)
    S["chunksB"] = chunksB
    # stream order template: (coh, r, t)
    orderB = []
    n_coh_B = (UT + COH - 1) // COH
    for co in range(n_coh_B):
        tiles = list(range(co * COH, min((co + 1) * COH, UT)))
        for r in range(NRM):
            for t in tiles:
                orderB.append((co, r, t, int(chunksB[r][t])))
    S["orderB"] = orderB
    S["NB"] = int(sum(o[3] for o in orderB))

    Bg, Bl = [], []
    for c in range(W):
        m = u_core == c
        key_t = b_tile[m]
        key_r = b_rng[m]
        key_d = b_gi[m]
        key_coh = key_t // COH
        o = np.lexsort((key_d, key_t, key_r, key_coh))
        st = key_t[o]
        sr = key_r[o]
        sd = key_d[o]
        sl = (u_loc[m][o] - st * P)
        # place into padded stream
        g = np.zeros(S["NB"] * P, np.int16)
        l = np.full(S["NB"] * P, SENT, np.float32)
        pos = 0
        ptr = 0
        for (co, r, t, nch) in orderB:
            n = int(cntB[c, r, t])
            g[pos:pos + n] = sd[ptr:ptr + n].astype(np.int16)
            l[pos:pos + n] = sl[ptr:ptr + n]
            ptr += n
            pos += nch * P
        assert ptr == m.sum()
        cntc = cntB[c]
        g = _mark_trailing_pads(g, orderB, lambda seg: (seg[0], seg[1]),
                                lambda seg: int(cntc[seg[1], seg[2]]))
        Bg.append(_wrap16(g))
        nbp = (S["NB"] + 3) // 4 * 4
        lp = np.full(nbp * P, SENT, np.float32)
        lp[:len(l)] = l
        Bl.append(_col_layout(lp, nbp, SENT, np.float32).astype(npbf16))
    S["b_gidx"] = Bg
    S["b_loc"] = Bl

    # ---- pass C: shard by dst core; order (cohort, r7, tile, src) ----
    m_core = ed_a // MSL
    m_loc = ed_a - m_core * MSL
    c_tile = m_loc // P
    _ur, _ul = _split_lut(NUP, USR, UH_A, UR)
    c_rng = _ur[es_a]
    c_gi = _ul[es_a]
    chunksC, cntC = _build_pass_streams(m_core, None, None, c_rng, c_tile, NRU, MT, COH)
    S["chunksC"] = chunksC
    orderC = []
    n_coh_C = (MT + COH - 1) // COH
    for co in range(n_coh_C):
        tiles = list(range(co * COH, min((co + 1) * COH, MT)))
        for r in range(NRU):
            for t in tiles:
                orderC.append((co, r, t, int(chunksC[r][t])))
    S["orderC"] = orderC
    S["NC"] = int(sum(o[3] for o in orderC))

    Cg, Cl = [], []
    for c in range(W):
        m = m_core == c
        key_t = c_tile[m]
        key_r = c_rng[m]
        key_s = c_gi[m]
        key_coh = key_t // COH
        o = np.lexsort((key_s, key_t, key_r, key_coh))
        st = key_t[o]
        sr = key_r[o]
        ss = key_s[o]
        sl = (m_loc[m][o] - st * P)
        g = np.zeros(S["NC"] * P, np.int16)
        l = np.full(S["NC"] * P, SENT, np.float32)
        pos = 0
        ptr = 0
        for (co, r, t, nch) in orderC:
            n = int(cntC[c, r, t])
            g[pos:pos + n] = ss[ptr:ptr + n].astype(np.int16)
            l[pos:pos + n] = sl[ptr:ptr + n]
            ptr += n
            pos += nch * P
        assert ptr == m.sum()
        cntc = cntC[c]
        g = _mark_trailing_pads(g, orderC, lambda seg: (seg[0], seg[1]),
                                lambda seg: int(cntc[seg[1], seg[2]]))
        Cg.append(_wrap16(g))
        ncp = (S["NC"] + 3) // 4 * 4
        lp = np.full(ncp * P, SENT, np.float32)
        lp[:len(l)] = l
        Cl.append(_col_layout(lp, ncp, SENT, np.float32).astype(npbf16))
    S["c_gidx"] = Cg
    S["c_loc"] = Cl

    # ---- pass D: labels by user core; order (r3, tile) ----
    l_core = lu // USR
    l_uloc = lu - l_core * USR
    d_tile = l_uloc // P
    _mr, _ml = _split_lut(NMP, MSL, MO_A, MR)
    d_rng = _mr[lm]
    d_gi = _ml[lm]
    cntD = np.zeros((W, NRD, UT), np.int64)
    np.add.at(cntD, (l_core, d_rng, d_tile), 1)
    mxD = cntD.max(axis=0)
    chunksD = (mxD + P - 1) // P
    S["chunksD"] = chunksD
    orderD = []
    for r in range(NRD):
        for t in range(UT):
            orderD.append((r, t, int(chunksD[r][t])))
    S["orderD"] = orderD
    S["ND"] = int(sum(o[2] for o in orderD))

    Dg, Du, Dreal = [], [], []
    for c in range(W):
        m = l_core == c
        idxs = np.nonzero(m)[0]
        kt = d_tile[m]
        kr = d_rng[m]
        o = np.lexsort((kt, kr))
        st = kt[o]
        sm = d_gi[m][o]
        sr = kr[o]
        sl = (l_uloc[m][o] - st * P)
        g = np.zeros(S["ND"] * P, np.int16)
        ul = np.full(S["ND"] * P, SENT, np.float32)
        real = np.full(S["ND"] * P, -1, np.int64)
        pos = 0
        ptr = 0
        for (r, t, nch) in orderD:
            n = int(cntD[c, r, t])
            g[pos:pos + n] = sm[ptr:ptr + n].astype(np.int16)
            ul[pos:pos + n] = sl[ptr:ptr + n]
            real[pos:pos + n] = idxs[o][ptr:ptr + n]
            ptr += n
            pos += nch * P
        assert ptr == m.sum()
        cntc = cntD[c]
        g = _mark_trailing_pads(g, orderD, lambda seg: seg[0],
                                lambda seg: int(cntc[seg[0], seg[1]]))
        Dg.append(_wrap16(g))
        Du.append(ul.astype(npbf16)[None, :])      # [1, ND*P] row layout
        Dreal.append(real)
    S["d_mgidx"] = Dg
    S["d_uloc"] = Du
    return S, Dreal


# ---------------- device program ----------------

def _gather_groups(order, key_fn):
    """Windows of <=GROUP chunks within contiguous key_fn spans.
    Returns list of (key, chunk0, gn)."""
    groups = []
    for key, c0, nch in _gather_spans(order, key_fn):
        gpos = 0
        while gpos < nch:
            gn = min(GROUP, nch - gpos)
            groups.append((key, c0 + gpos, gn))
            gpos += gn
    return groups


def _mark_trailing_pads(g, order, key_fn, seg_counts_for_core):
    return g  # trailing -1 idx crashed HW; keep idx-0 pads (zeroed by one-hot sentinel)

    """Set gather idx to -1 for pad slots with no real edge after them inside
    their gather group. g is the flat idx stream [NCH*128]."""
    seg_real = []
    pos = 0
    for seg in order:
        nch = seg[-1]
        n = seg_counts_for_core(seg)
        seg_real.append((pos, pos + n))
        pos += nch * P
    # real mask per slot
    real = np.zeros(len(g), bool)
    for a, b in seg_real:
        real[a:b] = True
    for key, c0, gn in _gather_groups(order, key_fn):
        a, b = c0 * P, (c0 + gn) * P
        w = real[a:b]
        nz = np.nonzero(w)[0]
        if len(nz) == 0:
            g[a] = 0          # keep one valid idx so the call isn't all-negative
            g[a + 1:b] = -1
            continue
        tail = a + int(nz[-1]) + 1
        g[tail:b] = -1        # everything after the last real slot is a pad
    return g


def _gather_spans(order, key_fn):
    """Group stream segments into gather spans keyed by key_fn((co,r,t)).
    Returns list of (key, chunk0, nchunks)."""
    spans = []
    pos = 0
    for seg in order:
        nch = seg[-1]
        k = key_fn(seg)
        if spans and spans[-1][0] == k:
            spans[-1] = (k, spans[-1][1], spans[-1][2] + nch)
        else:
            spans.append((k, pos, nch))
        pos += nch
    return spans


def build_program(S):
    nc = bacc.Bacc("TRN2", target_bir_lowering=False, debug=False, num_devices=W)
    NB, NC, ND = S["NB"], S["NC"], S["ND"]
    orderB, orderC, orderD = S["orderB"], S["orderC"], S["orderD"]

    # ---- kernel I/O ----
    featsT = nc.dram_tensor("featsT", [FD, MSL], bf16, kind="ExternalInput")
    wm4 = nc.dram_tensor("wm4", [FD, H], bf16, kind="ExternalInput")
    wnamesHH = ["wl1mu", "wr1um", "wl2mu", "wl2um", "wr2mu", "wr2um"]
    wt = {n: nc.dram_tensor(n, [H, H], bf16, kind="ExternalInput") for n in wnamesHH}
    bnames = ["bm_col", "b1_col", "bl1um_col", "b2_col"]
    bt = {n: nc.dram_tensor(n, [H], f32, kind="ExternalInput") for n in bnames}
    b3_row = nc.dram_tensor("b3_row", [1, H], bf16, kind="ExternalInput")
    recip_u = nc.dram_tensor("recip_u", [P, UT], f32, kind="ExternalInput")
    recip_m = nc.dram_tensor("recip_m", [P, MT], f32, kind="ExternalInput")
    zmask = nc.dram_tensor("zmask", [1, 1], f32, kind="ExternalInput")
    zfix = nc.dram_tensor("zfix", [1, 2 * H], bf16, kind="ExternalInput")
    iota_in = nc.dram_tensor("iota", [P, 4 * P], bf16, kind="ExternalInput")
    piota_in = nc.dram_tensor("piota", [P, P], f32, kind="ExternalInput")
    NBp = (NB + 3) // 4 * 4
    b_loc = nc.dram_tensor("b_loc", [P, NBp], bf16, kind="ExternalInput")
    b_gidx = nc.dram_tensor("b_gidx", [P, NB * 8], i16, kind="ExternalInput")
    NCp = (NC + 3) // 4 * 4
    c_loc = nc.dram_tensor("c_loc", [P, NCp], bf16, kind="ExternalInput")
    c_gidx = nc.dram_tensor("c_gidx", [P, NC * 8], i16, kind="ExternalInput")
    d_uloc = nc.dram_tensor("d_uloc", [1, ND * P], bf16, kind="ExternalInput")
    d_mgidx = nc.dram_tensor("d_mgidx", [P, ND * 8], i16, kind="ExternalInput")
    out = nc.dram_tensor("out", [P, ND], f32, kind="ExternalOutput")

    # ---- internal DRAM (split tables for overlapped AllGathers) ----
    xcat_slA = nc.dram_tensor("xcat_slA", [XC_A, 2 * H], bf16)
    xcat_slB = nc.dram_tensor("xcat_slB", [XC_B, 2 * H], bf16)
    xfA = nc.dram_tensor("xfA", [W * XC_A, 2 * H], bf16, addr_space="Shared")
    xfB = nc.dram_tensor("xfB", [W * XC_B, 2 * H], bf16, addr_space="Shared")
    userh_slA = nc.dram_tensor("userh_slA", [UH_A, H], bf16)
    userh_slB = nc.dram_tensor("userh_slB", [UH_B, H], bf16)
    uhfA = nc.dram_tensor("uhfA", [W * UH_A, H], bf16, addr_space="Shared")
    uhfB = nc.dram_tensor("uhfB", [W * UH_B, H], bf16, addr_space="Shared")
    mo_slA = nc.dram_tensor("mo_slA", [MO_A, H], bf16)
    mo_slB = nc.dram_tensor("mo_slB", [MO_B, H], bf16)
    mofA = nc.dram_tensor("mofA", [W * MO_A, H], bf16, addr_space="Shared")
    mofB = nc.dram_tensor("mofB", [W * MO_B, H], bf16, addr_space="Shared")

    rg = [list(range(W))]

    from contextlib import ExitStack
    with tile.TileContext(nc) as tc, ExitStack() as stack:
        cst = stack.enter_context(tc.tile_pool(name="cst", bufs=1))

        # ---------- constants ----------
        iota_t = cst.tile([P, 4 * P], bf16)
        nc.sync.dma_start(out=iota_t[:], in_=iota_in[:])
        piota_t = cst.tile([P, P], f32)
        nc.sync.dma_start(out=piota_t[:], in_=piota_in[:])
        ident_bf = cst.tile([P, P], bf16)
        make_identity(nc, ident_bf[:])
        ones1_bf = cst.tile([1, P], bf16)
        nc.vector.memset(ones1_bf[:], 1.0)
        ru_t = cst.tile([P, UT], f32)
        nc.sync.dma_start(out=ru_t[:], in_=recip_u[:])
        rm_t = cst.tile([P, MT], f32)
        nc.sync.dma_start(out=rm_t[:], in_=recip_m[:])
        zmask_t = cst.tile([1, 1], f32)
        nc.sync.dma_start(out=zmask_t[:], in_=zmask[:])
        zfix_t = cst.tile([1, 2 * H], bf16)
        nc.sync.dma_start(out=zfix_t[:], in_=zfix[:])
        b3_t = cst.tile([1, H], bf16)
        nc.sync.dma_start(out=b3_t[:], in_=b3_row[:])

        wtile = {}
        for n in wnamesHH:
            t = cst.tile([P, P], bf16, tag=f"w_{n}")
            nc.sync.dma_start(out=t[:], in_=wt[n][:])
            wtile[n] = t
        wm_t = []
        for k in range(4):
            t = cst.tile([P, H], bf16, tag=f"wm_{k}")
            nc.sync.dma_start(out=t[:], in_=wm4[k * P:(k + 1) * P, :])
            wm_t.append(t)
        bcol = {}
        for n in bnames:
            t = cst.tile([P, 1], f32, tag=f"b_{n}")
            nc.sync.dma_start(out=t[:], in_=bt[n][:, None])
            bcol[n] = t

        # mht resident [P, MSL] bf16 (~19.8KB/partition)
        mht = cst.tile([P, MSL], bf16)
        # uo resident [P, UT*H] bf16 (~49KB/partition)
        uo_sb = cst.tile([P, UT * H], bf16)

        # ---------- stage0: movie tables ----------
        NCT = (MSL + 511) // 512
        with nc.named_scope("stage0"), \
             tc.tile_pool(name="s0_ft", bufs=3) as s0_ft, \
             tc.tile_pool(name="s0_sb", bufs=2) as s0_sb, \
             tc.tile_pool(name="s0_ps", bufs=1, space="PSUM") as s0_ps, \
             tc.tile_pool(name="s0_pt", bufs=2, space="PSUM") as s0_pt, \
             tc.tile_pool(name="s0_stg", bufs=3) as s0_stg:
            for j in range(NCT):
                c0 = j * 512
                cw = min(512, MSL - c0)
                mxps = s0_ps.tile([P, 512], f32, space="PSUM", tag="mx")
                for k in range(4):
                    ft = s0_ft.tile([P, 512], bf16, tag="ft")
                    nc.sync.dma_start(out=ft[:, :cw], in_=featsT[k * P:(k + 1) * P, c0:c0 + cw])
                    nc.tensor.matmul(out=mxps[:, :cw], lhsT=wm_t[k][:], rhs=ft[:, :cw],
                                     start=(k == 0), stop=(k == 3))
                mxt = s0_sb.tile([P, 512], bf16, tag="mxt")
                nc.scalar.activation(out=mxt[:, :cw], in_=mxps[:, :cw], func=AF.Identity,
                                     bias=bcol["bm_col"][:])
                p1ps = s0_ps.tile([P, 512], f32, space="PSUM", tag="p1")
                nc.tensor.matmul(out=p1ps[:, :cw], lhsT=wtile["wl1mu"][:], rhs=mxt[:, :cw],
                                 start=True, stop=True)
                p1T = s0_sb.tile([P, 512], bf16, tag="p1T")
                nc.scalar.activation(out=p1T[:, :cw], in_=p1ps[:, :cw], func=AF.Identity,
                                     bias=bcol["b1_col"][:])
                aps = s0_ps.tile([P, 512], f32, space="PSUM", tag="A")
                nc.tensor.matmul(out=aps[:, :cw], lhsT=wtile["wr1um"][:], rhs=mxt[:, :cw],
                                 start=True, stop=True)
                nc.scalar.activation(out=mht[:, c0:c0 + cw], in_=aps[:, :cw], func=AF.Relu,
                                     bias=bcol["bl1um_col"][:])
                p2ps = s0_ps.tile([P, 512], f32, space="PSUM", tag="p2")
                nc.tensor.matmul(out=p2ps[:, :cw], lhsT=wtile["wl2mu"][:], rhs=mht[:, c0:c0 + cw],
                                 start=True, stop=True)
                p2T = s0_sb.tile([P, 512], bf16, tag="p2T")
                nc.scalar.activation(out=p2T[:, :cw], in_=p2ps[:, :cw], func=AF.Identity,
                                     bias=bcol["b2_col"][:])
                for tt in range(cw // 128):
                    gt = j * 4 + tt
                    xrow = s0_stg.tile([P, 2 * H], bf16, tag="xrow")
                    tpa = s0_pt.tile([P, P], bf16, space="PSUM", tag="tpa")
                    nc.tensor.transpose(out=tpa[:], in_=p1T[:, tt * P:(tt + 1) * P],
                                        identity=ident_bf[:])
                    nc.vector.tensor_copy(out=xrow[:, 0:H], in_=tpa[:])
                    tpb = s0_pt.tile([P, P], bf16, space="PSUM", tag="tpb")
                    nc.tensor.transpose(out=tpb[:], in_=p2T[:, tt * P:(tt + 1) * P],
                                        identity=ident_bf[:])
                    nc.scalar.copy(out=xrow[:, H:2 * H], in_=tpb[:])
                    if gt == ZROW_M % MSL // P and True:
                        # patch local row (ZROW_M%MSL)%P of this tile (slot 0 of tile 72)
                        nc.vector.scalar_tensor_tensor(
                            out=xrow[0:1, :], in0=xrow[0:1, :], scalar=zmask_t[0:1, 0:1],
                            in1=zfix_t[0:1, :], op0=mybir.AluOpType.mult,
                            op1=mybir.AluOpType.add)
                    if gt < XC_A // P:
                        nc.sync.dma_start(out=xcat_slA[gt * P:(gt + 1) * P, :], in_=xrow[:])
                    else:
                        gb_ = gt - XC_A // P
                        nc.sync.dma_start(out=xcat_slB[gb_ * P:(gb_ + 1) * P, :], in_=xrow[:])
                if j == (XC_A // P - 1) // 4:
                    nc.gpsimd.collective_compute(
                        "AllGather", mybir.AluOpType.bypass, replica_groups=rg,
                        ins=[xcat_slA[:].opt()], outs=[xfA[:].opt()])

        nc.gpsimd.collective_compute(
            "AllGather", mybir.AluOpType.bypass, replica_groups=rg,
            ins=[xcat_slB[:].opt()], outs=[xfB[:].opt()])

        # ---------- generic aggregation pass ----------
        def agg_pass(name, order, loc_dram, gidx_dram, n_stream, ranges,
                     width, n_rng, n_tiles, epilogue, emit_after_cohort=None):
            """order: list of (coh, r, t, nch). width: rhs free size (256 or 128)."""
            groupsAll = _gather_groups(order, key_fn=lambda seg: (seg[0], seg[1]))
            # chunk -> (group first chunk, tile index of buffer)
            with tc.tile_pool(name=f"{name}_sb", bufs=1) as sbp, \
                 tc.tile_pool(name=f"{name}_s4", bufs=4) as s4p, \
                 tc.tile_pool(name=f"{name}_g", bufs=3) as gp_, \
                 tc.tile_pool(name=f"{name}_gi", bufs=3) as gip, \
                 tc.tile_pool(name=f"{name}_acc", bufs=1, space="PSUM") as accp, \
                 tc.tile_pool(name=f"{name}_eps", bufs=1, space="PSUM") as epsp, \
                 tc.tile_pool(name=f"{name}_est", bufs=3) as estp:
                nsp = (n_stream + 3) // 4 * 4
                loc_t = sbp.tile([P, nsp], bf16)
                nc.sync.dma_start(out=loc_t[:], in_=loc_dram[:])

                # gather buffers, keyed by chunk
                gbufs = {}
                for _i in range(3):
                    zb = gp_.tile([P, GROUP * width], bf16, tag="gb", name="gb")
                    nc.vector.memset(zb[:], 0.0)

                def ensure_gathers(key):
                    for (k_, c0, gn) in groupsAll:
                        if k_ != key:
                            continue
                        r = k_[1]
                        tbl, rs, rl = ranges[r]
                        gb = gp_.tile([P, GROUP * width], bf16, tag="gb", name="gb")
                        gi = gip.tile([P, GROUP * 8], i16, tag="gi", name="gi")
                        col0 = c0 * 8
                        nc.sync.dma_start(out=gi[:, :gn * 8],
                                          in_=gidx_dram[:, col0: col0 + gn * 8])
                        nc.gpsimd.dma_gather(
                            out_ap=gb[:, :gn * width].rearrange("p (c n) -> p c n", c=gn),
                            in_ap=tbl[rs:rs + rl, :],
                            idxs_ap=gi[:, :gn * 8],
                            num_idxs=gn * P, num_idxs_reg=gn * P, elem_size=width)
                        for k in range(gn):
                            gbufs[c0 + k] = (gb, k)

                # s4 one-hot cache, keyed by 4-chunk group id
                s4cache = {}

                def get_s4(chunk):
                    g4 = chunk // 4
                    if g4 not in s4cache:
                        s4 = s4p.tile([P, 4 * P], bf16, tag="s4")
                        cc = g4 * 4
                        nc.vector.tensor_tensor(
                            out=s4[:].rearrange("p (k n) -> p k n", k=4),
                            in0=iota_t[:].rearrange("p (k n) -> p k n", k=4),
                            in1=loc_t[:, cc: cc + 4][:, :, None].to_broadcast([P, 4, P]),
                            op=mybir.AluOpType.is_equal)
                        s4cache[g4] = s4
                    return s4cache[g4]

                # per-cohort processing
                n_coh = (n_tiles + COH - 1) // COH
                # segment bookkeeping: positions in stream
                seg_pos = {}
                pos = 0
                for (co, r, t, nch) in order:
                    seg_pos[(r, t)] = (pos, nch)
                    pos += nch

                issued_spans = set()

                pos = 0
                oi = 0
                for co in range(n_coh):
                    tiles = list(range(co * COH, min((co + 1) * COH, n_tiles)))
                    accs = {t: accp.tile([P, width], f32, space="PSUM", name=f"acc{t}",
                                         tag=f"acc{t % COH}") for t in tiles}
                    # last (r, chunk) per tile for stop flag
                    last_chunk = {}
                    first_chunk = {}
                    for r in range(n_rng):
                        for t in tiles:
                            p0, nch = seg_pos[(r, t)]
                            if nch == 0:
                                continue
                            if t not in first_chunk:
                                first_chunk[t] = p0
                            last_chunk[t] = p0 + nch - 1
                    for r in range(n_rng):
                        if (co, r) not in issued_spans:
                            ensure_gathers((co, r))
                            issued_spans.add((co, r))
                        for t in tiles:
                            p0, nch = seg_pos[(r, t)]
                            for i in range(nch):
                                ch = p0 + i
                                gb, slot = gbufs[ch]
                                s4 = get_s4(ch)
                                nc.tensor.matmul(
                                    out=accs[t][:],
                                    lhsT=s4[:, (ch % 4) * P:(ch % 4 + 1) * P],
                                    rhs=gb[:, slot * width:(slot + 1) * width],
                                    start=(ch == first_chunk[t]),
                                    stop=(ch == last_chunk[t]))
                    for t in tiles:
                        epilogue(t, accs[t], epsp, estp)
                    if emit_after_cohort and co in emit_after_cohort:
                        emit_after_cohort[co]()

            return

        # ---------- pass B ----------
        def epilogue_B(t, acc, epsp, estp):
            rc = ru_t[:, t:t + 1]
            uh = estp.tile([P, H], bf16, tag="uh")
            nc.scalar.activation(out=uh[:], in_=acc[:, 0:H], func=AF.Relu, scale=rc)
            tp = epsp.tile([P, P], bf16, space="PSUM", tag="tp")
            nc.tensor.transpose(out=tp[:], in_=uh[:], identity=ident_bf[:])
            uht = estp.tile([P, P], bf16, tag="uht")
            nc.vector.tensor_copy(out=uht[:], in_=tp[:])
            psh = epsp.tile([P, H], f32, space="PSUM", tag="ps2")
            nc.tensor.matmul(out=psh[:], lhsT=uht[:], rhs=wtile["wl2um"][:],
                             start=True, stop=False)
            nc.tensor.matmul(out=psh[:], lhsT=ones1_bf[:], rhs=b3_t[:],
                             start=False, stop=True)
            uhsb = estp.tile([P, H], bf16, tag="uhsb")
            nc.vector.tensor_copy(out=uhsb[:], in_=psh[:])
            if t < UH_A // P:
                nc.sync.dma_start(out=userh_slA[t * P:(t + 1) * P, :], in_=uhsb[:])
            else:
                tb = t - UH_A // P
                nc.sync.dma_start(out=userh_slB[tb * P:(tb + 1) * P, :], in_=uhsb[:])
            psr = epsp.tile([P, H], f32, space="PSUM", tag="ps2")
            nc.tensor.matmul(out=psr[:], lhsT=uht[:], rhs=wtile["wr2mu"][:],
                             start=True, stop=True)
            uo1 = estp.tile([P, H], bf16, tag="uo1")
            nc.scalar.activation(out=uo1[:], in_=acc[:, H:2 * H], func=AF.Copy, scale=rc)
            nc.vector.tensor_tensor(out=uo_sb[:, t * H:(t + 1) * H], in0=uo1[:],
                                    in1=psr[:], op=mybir.AluOpType.add)

        RB = [(xfA, 0, 20480), (xfA, 20480, 20480),
              (xfB, 0, 19968), (xfB, 19968, 19968)]

        def ag_userh_A():
            nc.gpsimd.collective_compute(
                "AllGather", mybir.AluOpType.bypass, replica_groups=rg,
                ins=[userh_slA[:].opt()], outs=[uhfA[:].opt()])

        def ag_userh_B():
            nc.gpsimd.collective_compute(
                "AllGather", mybir.AluOpType.bypass, replica_groups=rg,
                ins=[userh_slB[:].opt()], outs=[uhfB[:].opt()])

        n_coh_B = (UT + COH - 1) // COH
        with nc.named_scope("passB"):
            agg_pass("pb", orderB, b_loc, b_gidx, NB, RB,
                     2 * H, NRM, UT, epilogue_B,
                     {UH_A // P // COH - 1: ag_userh_A, n_coh_B - 1: ag_userh_B})

        # ---------- pass C ----------
        def epilogue_C(t, acc, epsp, estp):
            rc = rm_t[:, t:t + 1]
            psr = epsp.tile([P, H], f32, space="PSUM", tag="psr")
            nc.tensor.matmul(out=psr[:], lhsT=mht[:, t * P:(t + 1) * P],
                             rhs=wtile["wr2um"][:], start=True, stop=True)
            mo1 = estp.tile([P, H], bf16, tag="mo1")
            nc.scalar.activation(out=mo1[:], in_=acc[:], func=AF.Copy, scale=rc)
            mo2 = estp.tile([P, H], bf16, tag="mo2")
            nc.vector.tensor_tensor(out=mo2[:], in0=mo1[:], in1=psr[:],
                                    op=mybir.AluOpType.add)
            if t < MO_A // P:
                nc.sync.dma_start(out=mo_slA[t * P:(t + 1) * P, :], in_=mo2[:])
            else:
                tb = t - MO_A // P
                nc.sync.dma_start(out=mo_slB[tb * P:(tb + 1) * P, :], in_=mo2[:])

        RU = [(uhfA, k * 24576, 24576) for k in range(4)] + \
             [(uhfB, k * 25600, 25600) for k in range(4)]

        def ag_mo_A():
            nc.gpsimd.collective_compute(
                "AllGather", mybir.AluOpType.bypass, replica_groups=rg,
                ins=[mo_slA[:].opt()], outs=[mofA[:].opt()])

        def ag_mo_B():
            nc.gpsimd.collective_compute(
                "AllGather", mybir.AluOpType.bypass, replica_groups=rg,
                ins=[mo_slB[:].opt()], outs=[mofB[:].opt()])

        n_coh_C = (MT + COH - 1) // COH
        with nc.named_scope("passC"):
            agg_pass("pc", orderC, c_loc, c_gidx, NC, RU,
                     H, NRU, MT, epilogue_C,
                     {MO_A // P // COH - 1: ag_mo_A, n_coh_C - 1: ag_mo_B})

        # ---------- pass D ----------
        with nc.named_scope("passD"), \
             tc.tile_pool(name="pd_sb", bufs=1) as pdsb, \
             tc.tile_pool(name="pd_ul", bufs=2) as pdul, \
             tc.tile_pool(name="pd_g", bufs=3) as pdg, \
             tc.tile_pool(name="pd_gi", bufs=3) as pdgi, \
             tc.tile_pool(name="pd_ps", bufs=3, space="PSUM") as pdps, \
             tc.tile_pool(name="pd_st", bufs=4) as pdst:
            outstrip = pdsb.tile([P, ND], f32)

            # gather spans: contiguous (r) ranges
            RD = [(mofA, 0, 21504), (mofA, 21504, 21504),
                  (mofB, 0, 18944), (mofB, 18944, 18944)]
            groupsD = _gather_groups(orderD, key_fn=lambda seg: seg[0])
            gbufsD = {}
            for _i in range(3):
                zb = pdg.tile([P, GROUP * H], bf16, tag="gb", name="gb")
                nc.vector.memset(zb[:], 0.0)

            def ensure_gathers_D(r):
                for (k_, c0, gn) in groupsD:
                    if k_ != r:
                        continue
                    gb = pdg.tile([P, GROUP * H], bf16, tag="gb", name="gb")
                    gi = pdgi.tile([P, GROUP * 8], i16, tag="gi", name="gi")
                    col0 = c0 * 8
                    nc.sync.dma_start(out=gi[:, :gn * 8],
                                      in_=d_mgidx[:, col0: col0 + gn * 8])
                    tbl, rs, rl = RD[r]
                    nc.gpsimd.dma_gather(
                        out_ap=gb[:, :gn * H].rearrange("p (c n) -> p c n", c=gn),
                        in_ap=tbl[rs:rs + rl, :],
                        idxs_ap=gi[:, :gn * 8],
                        num_idxs=gn * P, num_idxs_reg=gn * P, elem_size=H)
                    for k in range(gn):
                        gbufsD[c0 + k] = (gb, k)

            # uloc row pieces of 32 chunks
            ULW = 32

            ul_cache = {}

            def get_ul(chunk):
                blk = chunk // ULW
                if blk not in ul_cache:
                    w = min(ULW * P, ND * P - blk * ULW * P)
                    ul = pdul.tile([1, ULW * P], bf16, tag="ul")
                    nc.sync.dma_start(out=ul[0:1, :w],
                                      in_=d_uloc[0:1, blk * ULW * P: blk * ULW * P + w])
                    ul_cache[blk] = ul
                return ul_cache[blk]

            pos = 0
            issuedD = set()
            for (r, t, nch) in orderD:
                if r not in issuedD:
                    ensure_gathers_D(r)
                    issuedD.add(r)
                for i in range(nch):
                    ch = pos + i
                    gbm, slot = gbufsD[ch]
                    ul = get_ul(ch)
                    off = (ch % ULW) * P
                    psb = pdps.tile([P, P], f32, space="PSUM", tag="bc")
                    nc.tensor.matmul(out=psb[:], lhsT=ones1_bf[:],
                                     rhs=ul[0:1, off:off + P], start=True, stop=True)
                    E = pdst.tile([P, P], bf16, tag="E")
                    nc.vector.tensor_tensor(out=E[:], in0=piota_t[:], in1=psb[:],
                                            op=mybir.AluOpType.is_equal)
                    gu = pdps.tile([P, H], f32, space="PSUM", tag="gu")
                    nc.tensor.matmul(out=gu[:], lhsT=E[:],
                                     rhs=uo_sb[:, t * H:(t + 1) * H], start=True, stop=True)
                    scr = pdst.tile([P, H], bf16, tag="scr")
                    nc.vector.scalar_tensor_tensor(
                        out=scr[:], in0=gu[:], scalar=1.0,
                        in1=gbm[:, slot * H:(slot + 1) * H],
                        op0=mybir.AluOpType.mult, op1=mybir.AluOpType.mult,
                        accum_out=outstrip[:, ch:ch + 1])
                pos += nch
            nc.sync.dma_start(out=out[:], in_=outstrip[:])

    nc.compile()
    return nc


# ---------------- entry point ----------------

_CACHE = {}
TRACE = False
LAST_EXEC_NS = None
LAST_RESULTS = None


def kernel(movie_feats, user_init, edge_src, edge_dst, lbl_user, lbl_movie, n_users,
           Wm, bm,
           Wl1_um, bl1_um, Wr1_um, Wl1_mu, bl1_mu, Wr1_mu,
           Wl2_um, bl2_um, Wr2_um, Wl2_mu, bl2_mu, Wr2_mu):
    movie_feats = np.asarray(movie_feats, dtype=np.float32)
    u0 = np.asarray(user_init, np.float32)
    S, Dreal = preprocess(edge_src, edge_dst, lbl_user, lbl_movie)

    key = (S["NB"], S["NC"], S["ND"],
           S["chunksB"].tobytes(), S["chunksC"].tobytes(), S["chunksD"].tobytes())
    if key in _CACHE:
        nc = _CACHE[key]
    else:
        nc = build_program(S)
        _CACHE[key] = nc

    featsT = np.zeros((FD, NMP), npbf16)
    featsT[:, :NM] = movie_feats.T.astype(npbf16)

    # folded biases (host): B1 = bl1_mu + u0 @ Wr1_mu ; B2 = bl2_mu ; B3 = bl2_um
    B1 = (np.asarray(bl1_mu, np.float64) +
          u0.astype(np.float64) @ np.asarray(Wr1_mu, np.float64)).astype(np.float32)
    B2 = np.asarray(bl2_mu, np.float32)
    B3 = np.asarray(bl2_um, np.float32)

    iota_rep = np.tile(np.arange(P, dtype=np.float32)[None, :], (P, 4)).astype(npbf16)
    piota = np.tile(np.arange(P, dtype=np.float32)[:, None], (1, P))

    weights = {
        "wm4": np.asarray(Wm, np.float32).astype(npbf16),
        "wl1mu": np.asarray(Wl1_mu, np.float32).astype(npbf16),
        "wr1um": np.asarray(Wr1_um, np.float32).astype(npbf16),
        "wl2mu": np.asarray(Wl2_mu, np.float32).astype(npbf16),
        "wl2um": np.asarray(Wl2_um, np.float32).astype(npbf16),
        "wr2mu": np.asarray(Wr2_mu, np.float32).astype(npbf16),
        "wr2um": np.asarray(Wr2_um, np.float32).astype(npbf16),
        "bm_col": np.asarray(bm, np.float32),
        "b1_col": B1,
        "bl1um_col": np.asarray(bl1_um, np.float32),
        "b2_col": B2,
        "b3_row": B3.astype(npbf16)[None, :],
        "iota": iota_rep,
        "piota": piota.astype(np.float32),
    }

    zfix_row = np.concatenate([B1, B2]).astype(npbf16)[None, :]

    in_maps = []
    for c in range(W):
        m = {"featsT": np.ascontiguousarray(featsT[:, c * MSL:(c + 1) * MSL])}
        m.update(weights)
        m.update({
            "recip_u": S["recip_u"][c], "recip_m": S["recip_m"][c],
            "zmask": np.array([[0.0 if c == ZROW_M // MSL else 1.0]], np.float32),
            "zfix": zfix_row if c == ZROW_M // MSL else np.zeros((1, 2 * H), npbf16),
            "b_loc": S["b_loc"][c], "b_gidx": S["b_gidx"][c],
            "c_loc": S["c_loc"][c], "c_gidx": S["c_gidx"][c],
            "d_uloc": S["d_uloc"][c], "d_mgidx": S["d_mgidx"][c],
        })
        in_maps.append(m)

    global LAST_EXEC_NS, LAST_RESULTS
    res = run_bass_kernel_spmd(nc, in_maps, core_ids=list(range(W)), trace=TRACE)
    LAST_EXEC_NS = res.exec_time_ns
    LAST_RESULTS = res

    EL = len(np.asarray(lbl_user))
    out_full = np.zeros(EL, np.float32)
    for c in range(W):
        vals = res.results[c]["out"].T.reshape(-1)
        real = Dreal[c]
        mask = real >= 0
        out_full[real[mask]] = vals[mask]
    return out_full


# revision 18
# speedup vs baseline: 1.0698x; 1.0698x over previous
"""Trainium2 Bass kernel for nn_NovaLinkPredictor (hetero GraphSAGE link predictor).

8-core SPMD, v2:
  - Stage0: movie_x/movie_h/xcat tables per movie slice (bf16), biases folded into
    the tables (B1=bl1_mu+u0@Wr1_mu into p1, B2=bl2_mu into p2, B3=bl2_um into userh).
  - Pass B (user-side aggregation): edges sharded by src user range, ordered
    (cohort-of-10-user-tiles, movie-range, tile, dst); per-chunk one-hot scatter
    matmuls accumulate straight in PSUM across all 3 movie ranges (no SBUF acc).
    Degrees/recips come from host bincount; zero-degree users get fake edges to a
    patched [B1|B2] table row so mean+bias stays exact.
  - Pass C (movie-side aggregation): edges sharded by dst movie range; gathers the
    AllGather'd userh table (user_h @ Wl2_um + B3); direct local accumulation
    (no AllToAll / partials round-trip).
  - Pass D: labels sharded by user range; user_o rows come from SBUF-resident
    user_o via one-hot expansion matmuls (no user-side gather); movie rows via
    dma_gather of mo_full; dots via scalar_tensor_tensor accumulate.
  - Collectives: 3 AllGathers (xcat 5.2MB/rank, userh 6.4MB/rank, mo 2.6MB/rank).
"""
import sys
sys.path.insert(0, "/opt/trn_rl_repo")
import numpy as np
import ml_dtypes

from concourse import bass, mybir, bacc, tile
from concourse.bass_utils import run_bass_kernel_spmd
from concourse.masks import make_identity

# ---------------- constants ----------------
H = 128
NU = 200000
NM = 80000
FD = 512
W = 8
P = 128

USR = 25088            # users per core (196 tiles)
UT = 196
NUP = USR * W          # 200704
MSL = 10112            # movies per core (79 tiles)
MT = 79
NMP = MSL * W          # 80896

MRS = [0, 27008, 54016]            # movie-table int16 gather ranges
MRE = [27008, 54016, NMP]
NRM = 3
URSZ = 28672
URS = [k * URSZ for k in range(7)]  # user-table int16 gather ranges
NRU = 7
ZROW_M = 80000         # global movie row patched to [B1|B2] (core 7 local 9216)
ZROW_U = 200000        # global user row whose userh row is B3 (pad user)
SENT = 200.0
COH = 6                # user/movie tiles per psum cohort (6 PSUM banks + 2 epilogue)
GROUP = 8              # chunks per dma_gather call

bf16 = mybir.dt.bfloat16
f32 = mybir.dt.float32
i16 = mybir.dt.int16
npbf16 = ml_dtypes.bfloat16
AF = mybir.ActivationFunctionType


# ---------------- host-side preprocessing ----------------

def _wrap16(idx):
    n = idx.shape[0]
    assert n % 16 == 0
    w = idx.reshape(n // 16, 16).T.astype(np.int16)
    return np.ascontiguousarray(np.tile(w, (8, 1)))


def _col_layout(vals, n_chunks, fill, dtype):
    a = np.full(n_chunks * P, fill, dtype=dtype)
    a[: len(vals)] = vals
    return np.ascontiguousarray(a.reshape(n_chunks, P).T)


def _build_pass_streams(core_of, loc_of, gi_of, rng_of, tile_of, n_rng, n_tiles,
                        coh, min1_r0=True):
    """Generic (cohort, range, tile)-ordered stream builder.

    core_of/loc_of/gi_of/rng_of/tile_of: per-edge arrays (already restricted is
    done by caller via boolean masks per core).
    Returns (chunks[r][t] padded-to-max array, per-core (gidx_stream, loc_stream)).
    """
    cnt = np.zeros((W, n_rng, n_tiles), np.int64)
    np.add.at(cnt, (core_of, rng_of, tile_of), 1)
    mx = cnt.max(axis=0)
    chunks = (mx + P - 1) // P
    if min1_r0:
        chunks[0] = np.maximum(chunks[0], 1)
    return chunks, cnt


def _reg_counts(order, key_fn, cnt_for_core, n_cores):
    """Per gather-call num_idxs_reg: max over cores of (last real slot + 1)
    inside the call window, rounded up to 16. Skips trailing quantization pads
    (the ucode loops num_idxs_reg times)."""
    segs = []
    pos = 0
    for seg in order:
        nch = seg[-1]
        segs.append((seg, pos, nch * P))
        pos += nch * P
    counts = {}
    for key, c0, gn in _gather_groups(order, key_fn):
        a, b = c0 * P, (c0 + gn) * P
        overl = [(seg, spos, slen) for seg, spos, slen in segs
                 if spos < b and spos + slen > a]
        mx = 0
        for c in range(n_cores):
            endc = 0
            for seg, spos, slen in overl:
                n = cnt_for_core(c, seg)
                if n > 0:
                    endc = max(endc, min(spos + n, b) - a)
            mx = max(mx, endc)
        counts[c0] = min((max(mx, 1) + 15) // 16 * 16, gn * P)
    return counts


def preprocess(edge_src, edge_dst, lbl_user, lbl_movie):
    S = {}
    es = np.asarray(edge_src).astype(np.int64)
    ed = np.asarray(edge_dst).astype(np.int64)
    lu = np.asarray(lbl_user).astype(np.int64)
    lm = np.asarray(lbl_movie).astype(np.int64)

    udeg = np.bincount(es, minlength=NUP)
    mdeg = np.bincount(ed, minlength=NMP)
    recip_u = 1.0 / np.maximum(udeg, 1).astype(np.float64)
    recip_m = 1.0 / np.maximum(mdeg, 1).astype(np.float64)
    S["recip_u"] = [
        np.ascontiguousarray(
            recip_u[c * USR:(c + 1) * USR].reshape(UT, P).T.astype(np.float32))
        for c in range(W)]
    S["recip_m"] = [
        np.ascontiguousarray(
            recip_m[c * MSL:(c + 1) * MSL].reshape(MT, P).T.astype(np.float32))
        for c in range(W)]

    # fake edges: zero-degree users -> ZROW_M ([B1|B2] row); zero-deg movies -> ZROW_U
    fu = np.nonzero(udeg[:NU] == 0)[0]
    fm = np.nonzero(mdeg[:NM] == 0)[0]
    es_a = np.concatenate([es, fu, np.full(len(fm), ZROW_U, np.int64)])
    ed_a = np.concatenate([ed, np.full(len(fu), ZROW_M, np.int64), fm])

    # ---- pass B: shard by src core; order (cohort, r, tile, dst) ----
    u_core = es_a // USR
    u_loc = es_a - u_core * USR
    b_tile = u_loc // P
    b_rng = np.minimum(ed_a // 27008, 2)
    chunksB, cntB = _build_pass_streams(u_core, None, None, b_rng, b_tile, NRM, UT, COH)
    S["chunksB"] = chunksB
    # stream order template: (coh, r, t)
    orderB = []
    n_coh_B = (UT + COH - 1) // COH
    for co in range(n_coh_B):
        tiles = list(range(co * COH, min((co + 1) * COH, UT)))
        for r in range(NRM):
            for t in tiles:
                orderB.append((co, r, t, int(chunksB[r][t])))
    S["orderB"] = orderB
    S["NB"] = int(sum(o[3] for o in orderB))

    Bg, Bl = [], []
    for c in range(W):
        m = u_core == c
        key_t = b_tile[m]
        key_r = b_rng[m]
        key_d = ed_a[m]
        key_coh = key_t // COH
        o = np.lexsort((key_d, key_t, key_r, key_coh))
        st = key_t[o]
        sr = key_r[o]
        sd = key_d[o]
        sl = (u_loc[m][o] - st * P)
        # place into padded stream
        g = np.zeros(S["NB"] * P, np.int16)
        l = np.full(S["NB"] * P, SENT, np.float32)
        pos = 0
        ptr = 0
        for (co, r, t, nch) in orderB:
            n = int(cntB[c, r, t])
            g[pos:pos + n] = (sd[ptr:ptr + n] - MRS[r]).astype(np.int16)
            l[pos:pos + n] = sl[ptr:ptr + n]
            ptr += n
            pos += nch * P
        assert ptr == m.sum()
        cntc = cntB[c]
        g = _mark_trailing_pads(g, orderB, lambda seg: (seg[0], seg[1]),
                                lambda seg: int(cntc[seg[1], seg[2]]))
        Bg.append(_wrap16(g))
        nbp = (S["NB"] + 3) // 4 * 4
        lp = np.full(nbp * P, SENT, np.float32)
        lp[:len(l)] = l
        Bl.append(_col_layout(lp, nbp, SENT, np.float32).astype(npbf16))
    S["b_gidx"] = Bg
    S["b_loc"] = Bl
    S["b_regs"] = _reg_counts(orderB, lambda seg: (seg[0], seg[1]),
                              lambda c, seg: int(cntB[c, seg[1], seg[2]]), W)

    # ---- pass C: shard by dst core; order (cohort, r7, tile, src) ----
    m_core = ed_a // MSL
    m_loc = ed_a - m_core * MSL
    c_tile = m_loc // P
    c_rng = np.minimum(es_a // URSZ, NRU - 1)
    chunksC, cntC = _build_pass_streams(m_core, None, None, c_rng, c_tile, NRU, MT, COH)
    S["chunksC"] = chunksC
    orderC = []
    n_coh_C = (MT + COH - 1) // COH
    for co in range(n_coh_C):
        tiles = list(range(co * COH, min((co + 1) * COH, MT)))
        for r in range(NRU):
            for t in tiles:
                orderC.append((co, r, t, int(chunksC[r][t])))
    S["orderC"] = orderC
    S["NC"] = int(sum(o[3] for o in orderC))

    Cg, Cl = [], []
    for c in range(W):
        m = m_core == c
        key_t = c_tile[m]
        key_r = c_rng[m]
        key_s = es_a[m]
        key_coh = key_t // COH
        o = np.lexsort((key_s, key_t, key_r, key_coh))
        st = key_t[o]
        sr = key_r[o]
        ss = key_s[o]
        sl = (m_loc[m][o] - st * P)
        g = np.zeros(S["NC"] * P, np.int16)
        l = np.full(S["NC"] * P, SENT, np.float32)
        pos = 0
        ptr = 0
        for (co, r, t, nch) in orderC:
            n = int(cntC[c, r, t])
            g[pos:pos + n] = (ss[ptr:ptr + n] - URS[r]).astype(np.int16)
            l[pos:pos + n] = sl[ptr:ptr + n]
            ptr += n
            pos += nch * P
        assert ptr == m.sum()
        cntc = cntC[c]
        g = _mark_trailing_pads(g, orderC, lambda seg: (seg[0], seg[1]),
                                lambda seg: int(cntc[seg[1], seg[2]]))
        Cg.append(_wrap16(g))
        ncp = (S["NC"] + 3) // 4 * 4
        lp = np.full(ncp * P, SENT, np.float32)
        lp[:len(l)] = l
        Cl.append(_col_layout(lp, ncp, SENT, np.float32).astype(npbf16))
    S["c_gidx"] = Cg
    S["c_loc"] = Cl
    S["c_regs"] = _reg_counts(orderC, lambda seg: (seg[0], seg[1]),
                              lambda c, seg: int(cntC[c, seg[1], seg[2]]), W)

    # ---- pass D: labels by user core; order (r3, tile) ----
    l_core = lu // USR
    l_uloc = lu - l_core * USR
    d_tile = l_uloc // P
    d_rng = np.minimum(lm // 27008, 2)
    cntD = np.zeros((W, NRM, UT), np.int64)
    np.add.at(cntD, (l_core, d_rng, d_tile), 1)
    mxD = cntD.max(axis=0)
    chunksD = (mxD + P - 1) // P
    S["chunksD"] = chunksD
    orderD = []
    for r in range(NRM):
        for t in range(UT):
            orderD.append((r, t, int(chunksD[r][t])))
    S["orderD"] = orderD
    S["ND"] = int(sum(o[2] for o in orderD))

    Dg, Du, Dreal = [], [], []
    for c in range(W):
        m = l_core == c
        idxs = np.nonzero(m)[0]
        kt = d_tile[m]
        kr = d_rng[m]
        o = np.lexsort((kt, kr))
        st = kt[o]
        sm = lm[m][o]
        sr = kr[o]
        sl = (l_uloc[m][o] - st * P)
        g = np.zeros(S["ND"] * P, np.int16)
        ul = np.full(S["ND"] * P, SENT, np.float32)
        real = np.full(S["ND"] * P, -1, np.int64)
        pos = 0
        ptr = 0
        for (r, t, nch) in orderD:
            n = int(cntD[c, r, t])
            g[pos:pos + n] = (sm[ptr:ptr + n] - MRS[r]).astype(np.int16)
            ul[pos:pos + n] = sl[ptr:ptr + n]
            real[pos:pos + n] = idxs[o][ptr:ptr + n]
            ptr += n
            pos += nch * P
        assert ptr == m.sum()
        cntc = cntD[c]
        g = _mark_trailing_pads(g, orderD, lambda seg: seg[0],
                                lambda seg: int(cntc[seg[0], seg[1]]))
        Dg.append(_wrap16(g))
        Du.append(ul.astype(npbf16)[None, :])      # [1, ND*P] row layout
        Dreal.append(real)
    S["d_mgidx"] = Dg
    S["d_uloc"] = Du
    S["d_regs"] = _reg_counts(orderD, lambda seg: seg[0],
                              lambda c, seg: int(cntD[c, seg[0], seg[1]]), W)
    return S, Dreal


# ---------------- device program ----------------

def _gather_groups(order, key_fn):
    """Windows of <=GROUP chunks within contiguous key_fn spans.
    Returns list of (key, chunk0, gn)."""
    groups = []
    for key, c0, nch in _gather_spans(order, key_fn):
        gpos = 0
        while gpos < nch:
            gn = min(GROUP, nch - gpos)
            groups.append((key, c0 + gpos, gn))
            gpos += gn
    return groups


def _mark_trailing_pads(g, order, key_fn, seg_counts_for_core):
    return g

    """Set gather idx to -1 for pad slots with no real edge after them inside
    their gather group. g is the flat idx stream [NCH*128]."""
    seg_real = []
    pos = 0
    for seg in order:
        nch = seg[-1]
        n = seg_counts_for_core(seg)
        seg_real.append((pos, pos + n))
        pos += nch * P
    # real mask per slot
    real = np.zeros(len(g), bool)
    for a, b in seg_real:
        real[a:b] = True
    for key, c0, gn in _gather_groups(order, key_fn):
        a, b = c0 * P, (c0 + gn) * P
        w = real[a:b]
        nz = np.nonzero(w)[0]
        if len(nz) == 0:
            g[a] = 0          # keep one valid idx so the call isn't all-negative
            g[a + 1:b] = -1
            continue
        tail = a + int(nz[-1]) + 1
        g[tail:b] = -1        # everything after the last real slot is a pad
    return g


def _gather_spans(order, key_fn):
    """Group stream segments into gather spans keyed by key_fn((co,r,t)).
    Returns list of (key, chunk0, nchunks)."""
    spans = []
    pos = 0
    for seg in order:
        nch = seg[-1]
        k = key_fn(seg)
        if spans and spans[-1][0] == k:
            spans[-1] = (k, spans[-1][1], spans[-1][2] + nch)
        else:
            spans.append((k, pos, nch))
        pos += nch
    return spans


def build_program(S):
    nc = bacc.Bacc("TRN2", target_bir_lowering=False, debug=False, num_devices=W)
    NB, NC, ND = S["NB"], S["NC"], S["ND"]
    orderB, orderC, orderD = S["orderB"], S["orderC"], S["orderD"]

    # ---- kernel I/O ----
    featsT = nc.dram_tensor("featsT", [FD, MSL], bf16, kind="ExternalInput")
    wm4 = nc.dram_tensor("wm4", [FD, H], bf16, kind="ExternalInput")
    wnamesHH = ["wl1mu", "wr1um", "wl2mu", "wl2um", "wr2mu", "wr2um"]
    wt = {n: nc.dram_tensor(n, [H, H], bf16, kind="ExternalInput") for n in wnamesHH}
    bnames = ["bm_col", "b1_col", "bl1um_col", "b2_col"]
    bt = {n: nc.dram_tensor(n, [H], f32, kind="ExternalInput") for n in bnames}
    b3_row = nc.dram_tensor("b3_row", [1, H], bf16, kind="ExternalInput")
    recip_u = nc.dram_tensor("recip_u", [P, UT], f32, kind="ExternalInput")
    recip_m = nc.dram_tensor("recip_m", [P, MT], f32, kind="ExternalInput")
    zmask = nc.dram_tensor("zmask", [1, 1], f32, kind="ExternalInput")
    zfix = nc.dram_tensor("zfix", [1, 2 * H], bf16, kind="ExternalInput")
    iota_in = nc.dram_tensor("iota", [P, 4 * P], bf16, kind="ExternalInput")
    piota_in = nc.dram_tensor("piota", [P, P], f32, kind="ExternalInput")
    NBp = (NB + 3) // 4 * 4
    b_loc = nc.dram_tensor("b_loc", [P, NBp], bf16, kind="ExternalInput")
    b_gidx = nc.dram_tensor("b_gidx", [P, NB * 8], i16, kind="ExternalInput")
    NCp = (NC + 3) // 4 * 4
    c_loc = nc.dram_tensor("c_loc", [P, NCp], bf16, kind="ExternalInput")
    c_gidx = nc.dram_tensor("c_gidx", [P, NC * 8], i16, kind="ExternalInput")
    d_uloc = nc.dram_tensor("d_uloc", [1, ND * P], bf16, kind="ExternalInput")
    d_mgidx = nc.dram_tensor("d_mgidx", [P, ND * 8], i16, kind="ExternalInput")
    out = nc.dram_tensor("out", [P, ND], f32, kind="ExternalOutput")

    # ---- internal DRAM ----
    xcat_slice = nc.dram_tensor("xcat_slice", [MSL, 2 * H], bf16)
    xcat_full = nc.dram_tensor("xcat_full", [NMP, 2 * H], bf16, addr_space="Shared")
    userh_sl = nc.dram_tensor("userh_sl", [USR, H], bf16)
    userh_full = nc.dram_tensor("userh_full", [NUP, H], bf16, addr_space="Shared")
    mo_sl = nc.dram_tensor("mo_sl", [MSL, H], bf16)
    mo_full = nc.dram_tensor("mo_full", [NMP, H], bf16, addr_space="Shared")

    rg = [list(range(W))]

    from contextlib import ExitStack
    with tile.TileContext(nc) as tc, ExitStack() as stack:
        cst = stack.enter_context(tc.tile_pool(name="cst", bufs=1))

        # ---------- constants ----------
        iota_t = cst.tile([P, 4 * P], bf16)
        nc.sync.dma_start(out=iota_t[:], in_=iota_in[:])
        piota_t = cst.tile([P, P], f32)
        nc.sync.dma_start(out=piota_t[:], in_=piota_in[:])
        ident_bf = cst.tile([P, P], bf16)
        make_identity(nc, ident_bf[:])
        ones1_bf = cst.tile([1, P], bf16)
        nc.vector.memset(ones1_bf[:], 1.0)
        ru_t = cst.tile([P, UT], f32)
        nc.sync.dma_start(out=ru_t[:], in_=recip_u[:])
        rm_t = cst.tile([P, MT], f32)
        nc.sync.dma_start(out=rm_t[:], in_=recip_m[:])
        zmask_t = cst.tile([1, 1], f32)
        nc.sync.dma_start(out=zmask_t[:], in_=zmask[:])
        zfix_t = cst.tile([1, 2 * H], bf16)
        nc.sync.dma_start(out=zfix_t[:], in_=zfix[:])
        b3_t = cst.tile([1, H], bf16)
        nc.sync.dma_start(out=b3_t[:], in_=b3_row[:])

        wtile = {}
        for n in wnamesHH:
            t = cst.tile([P, P], bf16, tag=f"w_{n}")
            nc.sync.dma_start(out=t[:], in_=wt[n][:])
            wtile[n] = t
        wm_t = []
        for k in range(4):
            t = cst.tile([P, H], bf16, tag=f"wm_{k}")
            nc.sync.dma_start(out=t[:], in_=wm4[k * P:(k + 1) * P, :])
            wm_t.append(t)
        bcol = {}
        for n in bnames:
            t = cst.tile([P, 1], f32, tag=f"b_{n}")
            nc.sync.dma_start(out=t[:], in_=bt[n][:, None])
            bcol[n] = t

        # mht resident [P, MSL] bf16 (~19.8KB/partition)
        mht = cst.tile([P, MSL], bf16)
        # uo resident [P, UT*H] bf16 (~49KB/partition)
        uo_sb = cst.tile([P, UT * H], bf16)

        # ---------- stage0: movie tables ----------
        NCT = (MSL + 511) // 512
        with nc.named_scope("stage0"), \
             tc.tile_pool(name="s0_ft", bufs=3) as s0_ft, \
             tc.tile_pool(name="s0_sb", bufs=2) as s0_sb, \
             tc.tile_pool(name="s0_ps", bufs=1, space="PSUM") as s0_ps, \
             tc.tile_pool(name="s0_pt", bufs=2, space="PSUM") as s0_pt, \
             tc.tile_pool(name="s0_stg", bufs=3) as s0_stg:
            for j in range(NCT):
                c0 = j * 512
                cw = min(512, MSL - c0)
                mxps = s0_ps.tile([P, 512], f32, space="PSUM", tag="mx")
                for k in range(4):
                    ft = s0_ft.tile([P, 512], bf16, tag="ft")
                    nc.sync.dma_start(out=ft[:, :cw], in_=featsT[k * P:(k + 1) * P, c0:c0 + cw])
                    nc.tensor.matmul(out=mxps[:, :cw], lhsT=wm_t[k][:], rhs=ft[:, :cw],
                                     start=(k == 0), stop=(k == 3))
                mxt = s0_sb.tile([P, 512], bf16, tag="mxt")
                nc.scalar.activation(out=mxt[:, :cw], in_=mxps[:, :cw], func=AF.Identity,
                                     bias=bcol["bm_col"][:])
                p1ps = s0_ps.tile([P, 512], f32, space="PSUM", tag="p1")
                nc.tensor.matmul(out=p1ps[:, :cw], lhsT=wtile["wl1mu"][:], rhs=mxt[:, :cw],
                                 start=True, stop=True)
                p1T = s0_sb.tile([P, 512], bf16, tag="p1T")
                nc.scalar.activation(out=p1T[:, :cw], in_=p1ps[:, :cw], func=AF.Identity,
                                     bias=bcol["b1_col"][:])
                aps = s0_ps.tile([P, 512], f32, space="PSUM", tag="A")
                nc.tensor.matmul(out=aps[:, :cw], lhsT=wtile["wr1um"][:], rhs=mxt[:, :cw],
                                 start=True, stop=True)
                nc.scalar.activation(out=mht[:, c0:c0 + cw], in_=aps[:, :cw], func=AF.Relu,
                                     bias=bcol["bl1um_col"][:])
                p2ps = s0_ps.tile([P, 512], f32, space="PSUM", tag="p2")
                nc.tensor.matmul(out=p2ps[:, :cw], lhsT=wtile["wl2mu"][:], rhs=mht[:, c0:c0 + cw],
                                 start=True, stop=True)
                p2T = s0_sb.tile([P, 512], bf16, tag="p2T")
                nc.scalar.activation(out=p2T[:, :cw], in_=p2ps[:, :cw], func=AF.Identity,
                                     bias=bcol["b2_col"][:])
                for tt in range(cw // 128):
                    gt = j * 4 + tt
                    xrow = s0_stg.tile([P, 2 * H], bf16, tag="xrow")
                    tpa = s0_pt.tile([P, P], bf16, space="PSUM", tag="tpa")
                    nc.tensor.transpose(out=tpa[:], in_=p1T[:, tt * P:(tt + 1) * P],
                                        identity=ident_bf[:])
                    nc.vector.tensor_copy(out=xrow[:, 0:H], in_=tpa[:])
                    tpb = s0_pt.tile([P, P], bf16, space="PSUM", tag="tpb")
                    nc.tensor.transpose(out=tpb[:], in_=p2T[:, tt * P:(tt + 1) * P],
                                        identity=ident_bf[:])
                    nc.scalar.copy(out=xrow[:, H:2 * H], in_=tpb[:])
                    if gt == ZROW_M % MSL // P and True:
                        # patch local row (ZROW_M%MSL)%P of this tile (slot 0 of tile 72)
                        nc.vector.scalar_tensor_tensor(
                            out=xrow[0:1, :], in0=xrow[0:1, :], scalar=zmask_t[0:1, 0:1],
                            in1=zfix_t[0:1, :], op0=mybir.AluOpType.mult,
                            op1=mybir.AluOpType.add)
                    nc.sync.dma_start(out=xcat_slice[gt * P:(gt + 1) * P, :], in_=xrow[:])

        nc.gpsimd.collective_compute(
            "AllGather", mybir.AluOpType.bypass, replica_groups=rg,
            ins=[xcat_slice[:].opt()], outs=[xcat_full[:].opt()])

        # ---------- generic aggregation pass ----------
        def agg_pass(name, order, loc_dram, gidx_dram, n_stream, table_full, tstart,
                     tend, width, n_rng, n_tiles, epilogue, psum_tags, regs=None):
            """order: list of (coh, r, t, nch). width: rhs free size (256 or 128)."""
            groupsAll = _gather_groups(order, key_fn=lambda seg: (seg[0], seg[1]))
            # chunk -> (group first chunk, tile index of buffer)
            with tc.tile_pool(name=f"{name}_sb", bufs=1) as sbp, \
                 tc.tile_pool(name=f"{name}_s4", bufs=4) as s4p, \
                 tc.tile_pool(name=f"{name}_g", bufs=3) as gp_, \
                 tc.tile_pool(name=f"{name}_gi", bufs=3) as gip, \
                 tc.tile_pool(name=f"{name}_acc", bufs=1, space="PSUM") as accp, \
                 tc.tile_pool(name=f"{name}_eps", bufs=1, space="PSUM") as epsp, \
                 tc.tile_pool(name=f"{name}_est", bufs=3) as estp:
                nsp = (n_stream + 3) // 4 * 4
                loc_t = sbp.tile([P, nsp], bf16)
                nc.sync.dma_start(out=loc_t[:], in_=loc_dram[:])

                # gather buffers, keyed by chunk
                gbufs = {}
                for _i in range(3):
                    zb = gp_.tile([P, GROUP * width], bf16, tag="gb", name="gb")
                    nc.vector.memset(zb[:], 0.0)

                def ensure_gathers(key):
                    for (k_, c0, gn) in groupsAll:
                        if k_ != key:
                            continue
                        r = k_[1]
                        gb = gp_.tile([P, GROUP * width], bf16, tag="gb", name="gb")
                        gi = gip.tile([P, GROUP * 8], i16, tag="gi", name="gi")
                        col0 = c0 * 8
                        nc.sync.dma_start(out=gi[:, :gn * 8],
                                          in_=gidx_dram[:, col0: col0 + gn * 8])
                        nc.gpsimd.dma_gather(
                            out_ap=gb[:, :gn * width].rearrange("p (c n) -> p c n", c=gn),
                            in_ap=table_full[tstart[r]:tend[r], :],
                            idxs_ap=gi[:, :gn * 8],
                            num_idxs=gn * P,
                            num_idxs_reg=(regs.get(c0, gn * P) if regs else gn * P),
                            elem_size=width)
                        for k in range(gn):
                            gbufs[c0 + k] = (gb, k)

                # s4 one-hot cache, keyed by 4-chunk group id
                s4cache = {}

                def get_s4(chunk):
                    g4 = chunk // 4
                    if g4 not in s4cache:
                        s4 = s4p.tile([P, 4 * P], bf16, tag="s4")
                        cc = g4 * 4
                        nc.vector.tensor_tensor(
                            out=s4[:].rearrange("p (k n) -> p k n", k=4),
                            in0=iota_t[:].rearrange("p (k n) -> p k n", k=4),
                            in1=loc_t[:, cc: cc + 4][:, :, None].to_broadcast([P, 4, P]),
                            op=mybir.AluOpType.is_equal)
                        s4cache[g4] = s4
                    return s4cache[g4]

                # per-cohort processing
                n_coh = (n_tiles + COH - 1) // COH
                # segment bookkeeping: positions in stream
                seg_pos = {}
                pos = 0
                for (co, r, t, nch) in order:
                    seg_pos[(r, t)] = (pos, nch)
                    pos += nch

                issued_spans = set()

                pos = 0
                oi = 0
                for co in range(n_coh):
                    tiles = list(range(co * COH, min((co + 1) * COH, n_tiles)))
                    accs = {t: accp.tile([P, width], f32, space="PSUM", name=f"acc{t}",
                                         tag=f"acc{t % COH}") for t in tiles}
                    # last (r, chunk) per tile for stop flag
                    last_chunk = {}
                    first_chunk = {}
                    for r in range(n_rng):
                        for t in tiles:
                            p0, nch = seg_pos[(r, t)]
                            if nch == 0:
                                continue
                            if t not in first_chunk:
                                first_chunk[t] = p0
                            last_chunk[t] = p0 + nch - 1
                    for r in range(n_rng):
                        if (co, r) not in issued_spans:
                            ensure_gathers((co, r))
                            issued_spans.add((co, r))
                        for t in tiles:
                            p0, nch = seg_pos[(r, t)]
                            for i in range(nch):
                                ch = p0 + i
                                gb, slot = gbufs[ch]
                                s4 = get_s4(ch)
                                nc.tensor.matmul(
                                    out=accs[t][:],
                                    lhsT=s4[:, (ch % 4) * P:(ch % 4 + 1) * P],
                                    rhs=gb[:, slot * width:(slot + 1) * width],
                                    start=(ch == first_chunk[t]),
                                    stop=(ch == last_chunk[t]))
                    for t in tiles:
                        epilogue(t, accs[t], epsp, estp)
                        # release gather bufs for this cohort implicitly by pool reuse
                    # drop references so pools can recycle
                    for k in [k for k, v in list(gbufs.items())]:
                        pass

            return

        # ---------- pass B ----------
        def epilogue_B(t, acc, epsp, estp):
            rc = ru_t[:, t:t + 1]
            uh = estp.tile([P, H], bf16, tag="uh")
            nc.scalar.activation(out=uh[:], in_=acc[:, 0:H], func=AF.Relu, scale=rc)
            tp = epsp.tile([P, P], bf16, space="PSUM", tag="tp")
            nc.tensor.transpose(out=tp[:], in_=uh[:], identity=ident_bf[:])
            uht = estp.tile([P, P], bf16, tag="uht")
            nc.vector.tensor_copy(out=uht[:], in_=tp[:])
            psh = epsp.tile([P, H], f32, space="PSUM", tag="ps2")
            nc.tensor.matmul(out=psh[:], lhsT=uht[:], rhs=wtile["wl2um"][:],
                             start=True, stop=False)
            nc.tensor.matmul(out=psh[:], lhsT=ones1_bf[:], rhs=b3_t[:],
                             start=False, stop=True)
            uhsb = estp.tile([P, H], bf16, tag="uhsb")
            nc.vector.tensor_copy(out=uhsb[:], in_=psh[:])
            nc.sync.dma_start(out=userh_sl[t * P:(t + 1) * P, :], in_=uhsb[:])
            psr = epsp.tile([P, H], f32, space="PSUM", tag="ps2")
            nc.tensor.matmul(out=psr[:], lhsT=uht[:], rhs=wtile["wr2mu"][:],
                             start=True, stop=True)
            uo1 = estp.tile([P, H], bf16, tag="uo1")
            nc.scalar.activation(out=uo1[:], in_=acc[:, H:2 * H], func=AF.Copy, scale=rc)
            nc.vector.tensor_tensor(out=uo_sb[:, t * H:(t + 1) * H], in0=uo1[:],
                                    in1=psr[:], op=mybir.AluOpType.add)

        with nc.named_scope("passB"):
            agg_pass("pb", orderB, b_loc, b_gidx, NB, xcat_full, MRS, MRE,
                     2 * H, NRM, UT, epilogue_B, None, regs=S["b_regs"])

        nc.gpsimd.collective_compute(
            "AllGather", mybir.AluOpType.bypass, replica_groups=rg,
            ins=[userh_sl[:].opt()], outs=[userh_full[:].opt()])

        # ---------- pass C ----------
        def epilogue_C(t, acc, epsp, estp):
            rc = rm_t[:, t:t + 1]
            psr = epsp.tile([P, H], f32, space="PSUM", tag="psr")
            nc.tensor.matmul(out=psr[:], lhsT=mht[:, t * P:(t + 1) * P],
                             rhs=wtile["wr2um"][:], start=True, stop=True)
            mo1 = estp.tile([P, H], bf16, tag="mo1")
            nc.scalar.activation(out=mo1[:], in_=acc[:], func=AF.Copy, scale=rc)
            mo2 = estp.tile([P, H], bf16, tag="mo2")
            nc.vector.tensor_tensor(out=mo2[:], in0=mo1[:], in1=psr[:],
                                    op=mybir.AluOpType.add)
            nc.sync.dma_start(out=mo_sl[t * P:(t + 1) * P, :], in_=mo2[:])

        with nc.named_scope("passC"):
            agg_pass("pc", orderC, c_loc, c_gidx, NC, userh_full,
                     [URS[r] for r in range(NRU)],
                     [URS[r] + URSZ for r in range(NRU)],
                     H, NRU, MT, epilogue_C, None, regs=S["c_regs"])

        nc.gpsimd.collective_compute(
            "AllGather", mybir.AluOpType.bypass, replica_groups=rg,
            ins=[mo_sl[:].opt()], outs=[mo_full[:].opt()])

        # ---------- pass D ----------
        with nc.named_scope("passD"), \
             tc.tile_pool(name="pd_sb", bufs=1) as pdsb, \
             tc.tile_pool(name="pd_ul", bufs=2) as pdul, \
             tc.tile_pool(name="pd_g", bufs=3) as pdg, \
             tc.tile_pool(name="pd_gi", bufs=3) as pdgi, \
             tc.tile_pool(name="pd_ps", bufs=3, space="PSUM") as pdps, \
             tc.tile_pool(name="pd_st", bufs=4) as pdst:
            outstrip = pdsb.tile([P, ND], f32)

            # gather spans: contiguous (r) ranges
            groupsD = _gather_groups(orderD, key_fn=lambda seg: seg[0])
            gbufsD = {}
            for _i in range(3):
                zb = pdg.tile([P, GROUP * H], bf16, tag="gb", name="gb")
                nc.vector.memset(zb[:], 0.0)

            def ensure_gathers_D(r):
                for (k_, c0, gn) in groupsD:
                    if k_ != r:
                        continue
                    gb = pdg.tile([P, GROUP * H], bf16, tag="gb", name="gb")
                    gi = pdgi.tile([P, GROUP * 8], i16, tag="gi", name="gi")
                    col0 = c0 * 8
                    nc.sync.dma_start(out=gi[:, :gn * 8],
                                      in_=d_mgidx[:, col0: col0 + gn * 8])
                    nc.gpsimd.dma_gather(
                        out_ap=gb[:, :gn * H].rearrange("p (c n) -> p c n", c=gn),
                        in_ap=mo_full[MRS[r]:MRE[r], :],
                        idxs_ap=gi[:, :gn * 8],
                        num_idxs=gn * P,
                        num_idxs_reg=S["d_regs"].get(c0, gn * P), elem_size=H)
                    for k in range(gn):
                        gbufsD[c0 + k] = (gb, k)

            # uloc row pieces of 32 chunks
            ULW = 32

            ul_cache = {}

            def get_ul(chunk):
                blk = chunk // ULW
                if blk not in ul_cache:
                    w = min(ULW * P, ND * P - blk * ULW * P)
                    ul = pdul.tile([1, ULW * P], bf16, tag="ul")
                    nc.sync.dma_start(out=ul[0:1, :w],
                                      in_=d_uloc[0:1, blk * ULW * P: blk * ULW * P + w])
                    ul_cache[blk] = ul
                return ul_cache[blk]

            pos = 0
            issuedD = set()
            for (r, t, nch) in orderD:
                if r not in issuedD:
                    ensure_gathers_D(r)
                    issuedD.add(r)
                for i in range(nch):
                    ch = pos + i
                    gbm, slot = gbufsD[ch]
                    ul = get_ul(ch)
                    off = (ch % ULW) * P
                    psb = pdps.tile([P, P], f32, space="PSUM", tag="bc")
                    nc.tensor.matmul(out=psb[:], lhsT=ones1_bf[:],
                                     rhs=ul[0:1, off:off + P], start=True, stop=True)
                    E = pdst.tile([P, P], bf16, tag="E")
                    nc.vector.tensor_tensor(out=E[:], in0=piota_t[:], in1=psb[:],
                                            op=mybir.AluOpType.is_equal)
                    gu = pdps.tile([P, H], f32, space="PSUM", tag="gu")
                    nc.tensor.matmul(out=gu[:], lhsT=E[:],
                                     rhs=uo_sb[:, t * H:(t + 1) * H], start=True, stop=True)
                    scr = pdst.tile([P, H], bf16, tag="scr")
                    nc.vector.scalar_tensor_tensor(
                        out=scr[:], in0=gu[:], scalar=1.0,
                        in1=gbm[:, slot * H:(slot + 1) * H],
                        op0=mybir.AluOpType.mult, op1=mybir.AluOpType.mult,
                        accum_out=outstrip[:, ch:ch + 1])
                pos += nch
            nc.sync.dma_start(out=out[:], in_=outstrip[:])

    nc.compile()
    return nc


# ---------------- entry point ----------------

_CACHE = {}
TRACE = False
LAST_EXEC_NS = None
LAST_RESULTS = None


def kernel(movie_feats, user_init, edge_src, edge_dst, lbl_user, lbl_movie, n_users,
           Wm, bm,
           Wl1_um, bl1_um, Wr1_um, Wl1_mu, bl1_mu, Wr1_mu,
           Wl2_um, bl2_um, Wr2_um, Wl2_mu, bl2_mu, Wr2_mu):
    movie_feats = np.asarray(movie_feats, dtype=np.float32)
    u0 = np.asarray(user_init, np.float32)
    S, Dreal = preprocess(edge_src, edge_dst, lbl_user, lbl_movie)

    key = (S["NB"], S["NC"], S["ND"],
           S["chunksB"].tobytes(), S["chunksC"].tobytes(), S["chunksD"].tobytes())
    if key in _CACHE:
        nc = _CACHE[key]
    else:
        nc = build_program(S)
        _CACHE[key] = nc

    featsT = np.zeros((FD, NMP), npbf16)
    featsT[:, :NM] = movie_feats.T.astype(npbf16)

    # folded biases (host): B1 = bl1_mu + u0 @ Wr1_mu ; B2 = bl2_mu ; B3 = bl2_um
    B1 = (np.asarray(bl1_mu, np.float64) +
          u0.astype(np.float64) @ np.asarray(Wr1_mu, np.float64)).astype(np.float32)
    B2 = np.asarray(bl2_mu, np.float32)
    B3 = np.asarray(bl2_um, np.float32)

    iota_rep = np.tile(np.arange(P, dtype=np.float32)[None, :], (P, 4)).astype(npbf16)
    piota = np.tile(np.arange(P, dtype=np.float32)[:, None], (1, P))

    weights = {
        "wm4": np.asarray(Wm, np.float32).astype(npbf16),
        "wl1mu": np.asarray(Wl1_mu, np.float32).astype(npbf16),
        "wr1um": np.asarray(Wr1_um, np.float32).astype(npbf16),
        "wl2mu": np.asarray(Wl2_mu, np.float32).astype(npbf16),
        "wl2um": np.asarray(Wl2_um, np.float32).astype(npbf16),
        "wr2mu": np.asarray(Wr2_mu, np.float32).astype(npbf16),
        "wr2um": np.asarray(Wr2_um, np.float32).astype(npbf16),
        "bm_col": np.asarray(bm, np.float32),
        "b1_col": B1,
        "bl1um_col": np.asarray(bl1_um, np.float32),
        "b2_col": B2,
        "b3_row": B3.astype(npbf16)[None, :],
        "iota": iota_rep,
        "piota": piota.astype(np.float32),
    }

    zfix_row = np.concatenate([B1, B2]).astype(npbf16)[None, :]

    in_maps = []
    for c in range(W):
        m = {"featsT": np.ascontiguousarray(featsT[:, c * MSL:(c + 1) * MSL])}
        m.update(weights)
        m.update({
            "recip_u": S["recip_u"][c], "recip_m": S["recip_m"][c],
            "zmask": np.array([[0.0 if c == ZROW_M // MSL else 1.0]], np.float32),
            "zfix": zfix_row if c == ZROW_M // MSL else np.zeros((1, 2 * H), npbf16),
            "b_loc": S["b_loc"][c], "b_gidx": S["b_gidx"][c],
            "c_loc": S["c_loc"][c], "c_gidx": S["c_gidx"][c],
            "d_uloc": S["d_uloc"][c], "d_mgidx": S["d_mgidx"][c],
        })
        in_maps.append(m)

    global LAST_EXEC_NS, LAST_RESULTS
    res = run_bass_kernel_spmd(nc, in_maps, core_ids=list(range(W)), trace=TRACE)
    LAST_EXEC_NS = res.exec_time_ns
    LAST_RESULTS = res

    EL = len(np.asarray(lbl_user))
    out_full = np.zeros(EL, np.float32)
    for c in range(W):
        vals = res.results[c]["out"].T.reshape(-1)
        real = Dreal[c]
        mask = real >= 0
        out_full[real[mask]] = vals[mask]
    return out_full
